# revision 47
# baseline (speedup 1.0000x reference)
"""Contrastive-loss kernel for Trainium2, SPMD over 8 NeuronCores.

The reference loss over x[N=4, S=4096, F=256] is, for pairs a>b with
D[a,b] = ||x[:,a]-x[:,b]||^2 (summed over batch and feature):

    loss = [ sum_{a>b} D - sum_t D_sub[t] + sum_t relu(M - D_sub[t]) ]
           / (S*(S-1)*1000)
    sum_{a>b} D = S * sum x^2 - sum_{n,f} (sum_t x[n,t,f])^2
    D_sub[t]    = ||x[:,t+1]-x[:,t]||^2

Sharding: 8 cores = (batch n, seq half h).  Each core owns rows
[h*2048, (h+1)*2048) of one batch plus a halo row, staged host-side as a
contiguous [2049, 256] bf16 chunk (bf16 keeps every DVE op in 2x mode
and halves HBM traffic; the resulting ~1e-5 relative error is far under
the 2e-2 gate).  On device the chunk is loaded once as [128 partitions
x 17 rows x 256] (8.5KB-contiguous partition lines), so consecutive-row
differences are free-dim offsets inside a partition - no cross-partition
traffic.

Per row-group, pipelined behind 3 chunked loads:
  ACT : Square+accum partial sum(x^2) over rows 0..15, plus group 0's
        diff-square (ACT has slack there; it is off the DVE tail)
  PE  : ones-matmul column sums (c vector) accumulating in PSUM
  DVE : diff, diff^2, two folds over F (256 -> 64), all bf16 at the
        DVE's 2x rate
Outputs leave as soon as they exist (outC/outS and H rows 0:9
mid-stream on separate rings) so only the last fold's small DMA trails.
Host combines: folds the [128,16,64] partial sums, applies the margin
hinge over the 4095 consecutive-pair distances, and assembles the loss
in float64.
"""

import numpy as np
import ml_dtypes

import concourse.bass as bass
import concourse.tile as tile
from concourse import mybir
from concourse.bass_utils import run_bass_kernel_spmd

N, S, F = 4, 4096, 256
NCORES = 8
LOCAL = S // 2                 # 2048 rows per core (one batch, half the seq)
R = 16                         # rows per partition
CH = LOCAL + 1                 # staged rows (1-row halo; h=1 gets zeros)
MARGIN = 60000.0

# chunked load row-ranges (per partition) and the diff row groups
# (chunk 0 carries 5 rows: group 0's four diffs fill the DVE's wait
# for chunk 1 almost exactly)
CHUNKS = [(0, 5), (5, 10), (10, 17)]
GROUPS = [(0, 4), (4, 9), (9, 16)]
FOLD_W = 64                    # device folds F 256 -> 64; host does the rest
# The walrus build in this container cannot encode CUSTOM_DVE_ANT
# instructions ("ISA wrong length" in codegen), so the fused sq(a)+sq(b)
# op is unavailable; use the two-instruction mult+add fallback.
USE_SQPAIR = False

_program = None
TRACE = False
LAST_RESULT = None


def _patch_sem_clear():
    """The walrus build in this container cannot encode
    EVENT_SEMAPHORE_RANGE_CLEAR ("ISA wrong length" in codegen). Replace the
    tail range-clear that TileContext emits via Bass.clear_and_free_semaphores
    with per-semaphore EventSemaphore writes of 0 (sem-wr-imm), which the
    compiler does support."""
    import bass_rust
    from concourse.bass import compact_to_ranges

    if getattr(bass.Bass, "_sem_clear_patched", False):
        return

    def clear_and_free_semaphores(self, sems):
        if not sems:
            return
        sem_nums = [s.num if hasattr(s, "num") else s for s in sems]
        for sem_range in compact_to_ranges(sem_nums):
            assert self._state.free_isdisjoint(sem_range)
            self.gpsimd.dma_reset(sem_range)
            # Spread the clear writes over four sequencers so the ~12-sem
            # teardown chain runs in a quarter of the serial time (all
            # engines are quiescent here; write order across sems is free)
            engines = [self.sync, self.gpsimd, self.scalar, self.vector]
            for i, num in enumerate(sem_range):
                h = bass_rust.SemaphoreHandle(num=num, name=f"clr{num}")
                bi = engines[i % 4].sem_inc(h, 1)
                upd = bi.ins.sync_info.on_update[0]
                upd.update_mode = "sem-wr-imm"
                upd.update_value = 0
        self._state.prepend_free_semaphores(sem_nums)
        for poison_set in self._tile_sem_poison_stack:
            poison_set.update(sem_nums)

    bass.Bass.clear_and_free_semaphores = clear_and_free_semaphores
    bass.Bass._sem_clear_patched = True


def _split_multi_waits(nc: bass.Bass) -> None:
    """The walrus build here encodes at most ONE sync wait per instruction.
    Hoist surplus waits into standalone wait-only EventSemaphore instructions
    placed immediately before the owner on the same engine queue — semantics
    are identical (same queue, in-order), and every instruction ends up with
    a single wait."""
    import bass_rust

    wid = 0
    for b in nc.m.functions[0].blocks:
        out = []
        changed = False
        for inst in b.instructions:
            si = inst.sync_info
            waits = list(si.on_wait) if si is not None else []
            if len(waits) > 1:
                changed = True
                for w in waits[:-1]:
                    ev = bass_rust.InstEventSemaphore(
                        name=f"WSPLIT-{wid}", engine=inst.engine, ins=[], outs=[]
                    )
                    wid += 1
                    ev.sync_info = bass_rust.SyncInfo(on_wait=[w], on_update=[])
                    out.append(ev)
                inst.sync_info = bass_rust.SyncInfo(
                    on_wait=[waits[-1]], on_update=list(si.on_update)
                )
            out.append(inst)
        if changed:
            b.instructions = out


_SQPAIR = None


def _get_sqpair():
    """Register a custom DVE op  out = in0^2 + in1^2  (one pass, fuses the
    diff-square with the first fold over the feature dim)."""
    global _SQPAIR
    if _SQPAIR is not None:
        return _SQPAIR
    from concourse import dve_ops
    from concourse.dve_spec import Spec, Src0, Src1, sq

    def _ref(in0, in1, s0, s1, imm2):
        return in0.astype(np.float32) ** 2 + in1.astype(np.float32) ** 2

    op = dve_ops.DveOp(
        "SQPAIR_ANT",
        Spec(body=sq(Src0) + sq(Src1), reference=_ref),
        subdim=False,
        uops_sha={"v3": "cd4bd6e1c27efd14", "v4": "121e32d8332f5047"},
    )
    if op.name not in dve_ops._SUB_OPCODE_FOR_NAME:
        dve_ops.OPS.append(op)
        dve_ops.CUSTOM_DVE_SPECS[op.name] = op.spec
        dve_ops._SUB_OPCODE_FOR_NAME[op.name] = (
            dve_ops._CUSTOM_DVE_ROW_BASE + len(dve_ops.OPS) - 1
        )
    _SQPAIR = op
    return op


def _build_program() -> bass.Bass:
    _patch_sem_clear()
    f32 = mybir.dt.float32
    bf16 = mybir.dt.bfloat16
    nc = bass.Bass()
    xc = nc.declare_dram_parameter("xc", [CH, F], bf16, isOutput=False)
    # outF carries H rows 0:16 plus an extras row 16 (the three bf16
    # partial-sum(x^2) columns), so no separate outS DMA trails the kernel
    outF = nc.declare_dram_parameter(
        "outF", [128, (R + 1) * FOLD_W], bf16, isOutput=True
    )
    outC = nc.declare_dram_parameter("outC", [1, F], f32, isOutput=True)

    with tile.TileContext(nc) as tc:
        with (
            tc.tile_pool(name="data", bufs=1) as data,
            tc.tile_pool(name="small", bufs=1) as small,
            tc.tile_pool(name="psum", bufs=1, space="PSUM") as psum,
        ):
            X = data.tile([128, 17, F], bf16, tag="X")
            # Chunk loads first, on the sync ring: nothing else should gate
            # descriptor generation for the first byte.
            xc_base = xc[:, :]
            for a, b in CHUNKS:
                src = bass.AP(
                    tensor=xc_base.tensor,
                    offset=a * F,
                    ap=[[R * F, 128], [1, (b - a) * F]],
                )
                nc.sync.dma_start(out=X[:, a:b, :], in_=src)

            D = data.tile([128, R, F], bf16, tag="D")
            Dsq = data.tile([128, R, F], bf16, tag="Dsq")
            G = data.tile([128, R, 128], bf16, tag="G")
            H = data.tile([128, R + 1, FOLD_W], bf16, tag="H")
            junk = data.tile([128, 6, F], bf16, tag="junk")

            ssb = small.tile([128, 3], f32)
            onesb = small.tile([128, 1], bf16)
            nc.vector.memset(onesb, 1.0)
            # Warm the Square activation table while the first chunk loads,
            # so the 1.3us table load is off the critical path.
            warm = small.tile([128, 1], f32)
            nc.scalar.activation(
                out=warm, in_=warm, func=mybir.ActivationFunctionType.Square
            )

            pc = psum.tile([1, F], f32)
            sqpair = _get_sqpair() if USE_SQPAIR else None

            cb = small.tile([1, F], f32)
            # sum(x^2) row groups: last one finishes as early as possible
            # after chunk 2 lands so outS never sits in the tail
            SS = [(0, 4), (4, 10), (10, 16)]

            def fold(ra, rb):
                nc.vector.tensor_add(
                    G[:, ra:rb, :],
                    Dsq[:, ra:rb, 0:128],
                    Dsq[:, ra:rb, 128:256],
                )
                nc.vector.tensor_add(
                    H[:, ra:rb, :], G[:, ra:rb, 0:64], G[:, ra:rb, 64:128]
                )

            for k, ((a, b), (ra, rb)) in enumerate(zip(CHUNKS, GROUPS)):
                sa, sb = SS[k]
                nc.scalar.activation(
                    out=junk[:, 0 : sb - sa, :],
                    in_=X[:, sa:sb, :],
                    func=mybir.ActivationFunctionType.Square,
                    accum_out=ssb[:, k : k + 1],
                )

                # column sums for the Gram correction, accumulated in PSUM
                for r in range(ra, rb):
                    nc.tensor.matmul(
                        pc,
                        onesb,
                        X[:, r, :],
                        start=(r == 0),
                        stop=(r == 15),
                    )

                if k == 2:
                    # PSUM -> SBUF staging for c on ACT (off the saturated
                    # DVE stream; ACT has slack after its last square),
                    # and the sum(x^2) columns ride H's extras row
                    nc.scalar.copy(cb, pc)
                    nc.scalar.copy(H[:, R, 0:3], ssb)
                    nc.sync.dma_start(out=outC[:, :], in_=cb)

                # consecutive-row differences on DVE; group 0's square goes
                # to ACT (it is off the tail there), the rest stay on DVE
                nc.vector.tensor_sub(
                    D[:, ra:rb, :], X[:, ra + 1 : rb + 1, :], X[:, ra:rb, :]
                )
                if k == 0:
                    nc.scalar.activation(
                        out=Dsq[:, ra:rb, :],
                        in_=D[:, ra:rb, :],
                        func=mybir.ActivationFunctionType.Square,
                    )
                else:
                    nc.vector.tensor_mul(
                        Dsq[:, ra:rb, :], D[:, ra:rb, :], D[:, ra:rb, :]
                    )
                if k == 1:
                    # folds for groups 0 and 1 as one op pair (group 0's
                    # squares come from ACT, group 1's from DVE) while
                    # chunk 2 streams
                    fold(0, 9)
                elif k == 2:
                    # H rows 0:9 leave on the scalar ring once ACT drains
                    nc.scalar.dma_start(
                        out=outF[:, 0 : 9 * FOLD_W], in_=H[:, 0:9, :]
                    )
                    fold(9, 16)
                    nc.sync.dma_start(
                        out=outF[:, 9 * FOLD_W :], in_=H[:, 9 : R + 1, :]
                    )
    _split_multi_waits(nc)
    return nc


def _get_program() -> bass.Bass:
    global _program
    if _program is None:
        _program = _build_program()
    return _program


def _to_f32(arr: np.ndarray) -> np.ndarray:
    """bf16 (ml_dtypes or uint16 view) -> float32, exactly."""
    if arr.dtype == np.float32:
        return arr
    if arr.dtype == np.uint16:
        return (arr.astype(np.uint32) << 16).view(np.float32)
    return arr.astype(np.float32)


def kernel(**inputs) -> np.ndarray:
    global LAST_RESULT
    x = np.ascontiguousarray(np.asarray(inputs["x"], dtype=np.float32))
    assert x.shape == (N, S, F)
    nc = _get_program()

    xb = x.astype(ml_dtypes.bfloat16)
    in_maps = []
    for k in range(NCORES):
        n, h = k // 2, k % 2
        t0 = h * LOCAL
        take = min(CH, S - t0)
        chunk = np.zeros((CH, F), dtype=ml_dtypes.bfloat16)
        chunk[:take] = xb[n, t0 : t0 + take]
        in_maps.append({"xc": chunk})

    LAST_RESULT = run_bass_kernel_spmd(
        nc, in_maps, list(range(NCORES)), trace=TRACE
    )
    res = LAST_RESULT.results

    ssum = 0.0
    gsum = 0.0
    c_by_n = [np.zeros(F, dtype=np.float64) for _ in range(N)]
    # D_half[h][t] accumulates ||x[:,t+1]-x[:,t]||^2 over the 4 batches
    D_half = [np.zeros(LOCAL, dtype=np.float64) for _ in range(2)]
    for k in range(NCORES):
        n, h = k // 2, k % 2
        r = res[k]
        c_by_n[n] += r["outC"][0].astype(np.float64)
        Hm = _to_f32(np.asarray(r["outF"])).astype(np.float64)
        Hm = Hm.reshape(128, R + 1, FOLD_W)
        # rows 0:16: H[p, r*FOLD_W + j], pair local index t = 16p + r;
        # row 16 cols 0:3: the partial sum(x^2) columns in bf16
        ssum += float(np.sum(Hm[:, R, 0:3]))
        D_half[h] += Hm[:, 0:R, :].sum(axis=2).reshape(-1)

    for n in range(N):
        gsum += float(np.sum(c_by_n[n] * c_by_n[n]))

    Dfull = np.concatenate([D_half[0], D_half[1][:-1]])  # pairs t=0..4094
    dsum = float(np.sum(Dfull))
    hsum = float(np.sum(np.maximum(0.0, MARGIN - Dfull)))

    numerator = S * ssum - gsum - dsum + hsum
    loss = numerator / float(S * (S - 1) * 1000)
    return np.asarray(loss, dtype=np.float32)
